# revision 19
# baseline (speedup 1.0000x reference)
"""Trainium2 8-core kernel for nn_Attention_68341519614426.

Reference computation (B=4, N=2048, D=1024, H=16, pd=64):
    qkv = x @ Wqkv.T + bqkv                       # (B, N, 3D)
    q, k, v = split/reshape -> (B, H, N, pd)
    att = softmax(q @ k.T)  (NO 1/sqrt(pd) scale)
    out = (att @ v)  reshaped (B,H,N,pd) -> (B,N,D) with NO transpose,
    i.e. each (b, h) head's flattened (N, pd) block is a contiguous chunk
    of the output.  => 64 fully independent (b, h) problems.

Sharding: 8 cores = 4 batches x 2 head-groups (8 heads each).  Pure data
parallel, no collectives.  Host pre-transposes/casts inputs; device does
QKV projection, scores, softmax (exp + fused denominator via an appended
ones-column of V), att@v, transpose back to token-major, normalization.

Device dataflow (per core, per head):
  qT,kT  : feature-major  [pd-feat (partitions), tokens]   (bf16)
  v_aug  : token-major    [tokens (partitions), 64 v feats + ones col]
  S^T    : [m key-tokens (128-chunk partitions), n query-tokens] psum
  E^T=exp(S^T) -> sbuf bf16
  O_aug^T[65, n-half] (psum) = sum_m v_aug[m].T @ E^T[m]  (row 64 = denom)
  PE-transpose 128-blocks -> [128 tok, 65], DVE divide by denom, DMA out.

Scheduling notes (HW-measured on trn2):
  * The PE HAM clock gate re-throttles 2.4 -> 1.2 GHz after even a
    ~1.4 us PE idle gap, and does NOT recover under attention's
    half-array (K=64 / M=65) matmul stream.  So the PE queue must never
    run dry: q/k projection matmuls for later head-pairs (and junk
    full-array matmuls once those run out) are interleaved 1-2 per
    inner-loop iteration, and each n-half's flush/transpose/normalize
    epilogue is deferred into the next half's iterations.
  * The att@v matmuls for iteration i are emitted during iteration i+1
    (software pipeline): the in-order PE queue must not park an
    exp-dependent matmul in front of the next scores matmul, or the
    ScalarE exp stream serializes with the PE.
"""

import os
import sys
from collections import deque

import numpy as np

if "/opt/trn_rl_repo" not in sys.path:
    sys.path.insert(0, "/opt/trn_rl_repo")

import ml_dtypes

import concourse.bass as bass
import concourse.tile as tile
from concourse import bacc, mybir
from concourse.bass_utils import run_bass_kernel_spmd
from concourse.masks import make_identity

BF16 = ml_dtypes.bfloat16

B, N, D = 4, 2048, 1024
H = 16
PD = 64
HEADS_PER_CORE = 8  # 2-way head parallel x 4-way batch parallel
SHARD_F = HEADS_PER_CORE * PD  # 512 q (or k, or v) features per core

_CACHE = {}


def _build_nc() -> bass.Bass:
    f32 = mybir.dt.float32
    bf16 = mybir.dt.bfloat16

    nc = bacc.Bacc()
    xt_h = nc.declare_dram_parameter("xt", [D, N], bf16, isOutput=False)
    wt_h = nc.declare_dram_parameter("wt", [D, 3 * SHARD_F], bf16, isOutput=False)
    bqk_h = nc.declare_dram_parameter("bias_qk", [128, 8], f32, isOutput=False)
    bv_h = nc.declare_dram_parameter(
        "bias_v", [128, HEADS_PER_CORE, PD], f32, isOutput=False
    )
    out_h = nc.declare_dram_parameter(
        "out", [HEADS_PER_CORE, N, PD], f32, isOutput=True
    )

    KC = D // 128  # 8 contraction chunks for the QKV projection
    NT512 = N // 512  # 4
    MCH = N // 128  # 16 key-token chunks
    QC = SHARD_F // 128  # 4 feature chunks for q (and for k)

    with tile.TileContext(nc) as tc:
        with (
            tc.tile_pool(name="consts", bufs=1) as consts,
            tc.tile_pool(name="big", bufs=1) as big,
            tc.tile_pool(name="ps", bufs=2, space="PSUM") as ps,
            tc.tile_pool(name="ops", bufs=1, space="PSUM") as ops,
            tc.tile_pool(name="qkvps", bufs=1, space="PSUM") as qkvps,
            tc.tile_pool(name="tpp", bufs=1, space="PSUM") as tpp,
            tc.tile_pool(name="epool", bufs=3) as epool,
            tc.tile_pool(name="osb", bufs=2) as osb,
            tc.tile_pool(name="outp", bufs=2) as outp,
            tc.tile_pool(name="small", bufs=4) as small,
        ):
            # ---- constants / inputs resident in SBUF ----
            bqk_sb = consts.tile([128, 8], f32, tag="bqk")
            nc.sync.dma_start(out=bqk_sb, in_=bqk_h[:])
            bv_sb = consts.tile([128, HEADS_PER_CORE, PD], f32, tag="bv")
            nc.sync.dma_start(out=bv_sb, in_=bv_h[:])
            ident = consts.tile([65, 65], bf16, tag="ident")
            make_identity(nc, ident)

            # per-chunk input DMAs: spread across DMA engines so the
            # first projection matmuls start ~2us in instead of waiting on
            # one serialized multi-MB transfer
            xt_sb = big.tile([128, KC, N], bf16, tag="xt")
            wt_sb = big.tile([128, KC, 3 * SHARD_F], bf16, tag="wt")
            for kc in range(KC):
                nc.sync.dma_start(
                    out=wt_sb[:, kc, 2 * SHARD_F : 3 * SHARD_F],
                    in_=wt_h[kc * 128 : (kc + 1) * 128, 2 * SHARD_F : 3 * SHARD_F],
                )
                nc.sync.dma_start(
                    out=xt_sb[:, kc, :], in_=xt_h[kc * 128 : (kc + 1) * 128, :]
                )
            for kc in range(KC):
                nc.sync.dma_start(
                    out=wt_sb[:, kc, 0 : 2 * SHARD_F],
                    in_=wt_h[kc * 128 : (kc + 1) * 128, 0 : 2 * SHARD_F],
                )

            qt_sb = big.tile([128, QC, N], bf16, tag="qt")
            kt_sb = big.tile([128, QC, N], bf16, tag="kt")
            vaug_sb = big.tile([128, MCH, HEADS_PER_CORE, PD + 1], bf16, tag="vaug")
            nc.vector.memset(vaug_sb[:, :, :, PD : PD + 1], 1.0)

            def emit_qk_tile(fc, t5):
                """One q/k projection psum tile: 8 matmuls + bias drain."""
                dst = qt_sb if fc < QC else kt_sb
                cc = fc % QC
                pt = ps.tile([128, 512], f32, tag="ps")
                for kc in range(KC):
                    nc.tensor.matmul(
                        pt,
                        lhsT=wt_sb[:, kc, fc * 128 : (fc + 1) * 128],
                        rhs=xt_sb[:, kc, t5 * 512 : (t5 + 1) * 512],
                        start=(kc == 0),
                        stop=(kc == KC - 1),
                    )
                nc.vector.tensor_scalar_add(
                    dst[:, cc, t5 * 512 : (t5 + 1) * 512],
                    pt,
                    bqk_sb[:, fc : fc + 1],
                )

            def qk_mm_gen(chunks):
                """Generator: one q/k projection matmul per next() call."""
                for c in chunks:
                    for fc in (c, QC + c):  # q chunk c, then k chunk c
                        dst = qt_sb if fc < QC else kt_sb
                        cc = fc % QC
                        for t5 in range(NT512):
                            pt = qkvps.tile([128, 512], f32, tag="qkv")
                            for kc in range(KC):
                                nc.tensor.matmul(
                                    pt,
                                    lhsT=wt_sb[:, kc, fc * 128 : (fc + 1) * 128],
                                    rhs=xt_sb[:, kc, t5 * 512 : (t5 + 1) * 512],
                                    start=(kc == 0),
                                    stop=(kc == KC - 1),
                                )
                                if kc == KC - 1:
                                    nc.vector.tensor_scalar_add(
                                        dst[:, cc, t5 * 512 : (t5 + 1) * 512],
                                        pt,
                                        bqk_sb[:, fc : fc + 1],
                                    )
                                yield True

            # ---- stage 1 preamble: v projection (token-major) + qk chunk 0 ----
            with nc.named_scope("qkv_preamble"):
                for tk in range(MCH):
                    pt = ps.tile([128, 512], f32, tag="ps")
                    for kc in range(KC):
                        nc.tensor.matmul(
                            pt,
                            lhsT=xt_sb[:, kc, tk * 128 : (tk + 1) * 128],
                            rhs=wt_sb[:, kc, 2 * SHARD_F : 3 * SHARD_F],
                            start=(kc == 0),
                            stop=(kc == KC - 1),
                        )
                    nc.vector.tensor_add(
                        vaug_sb[:, tk, :, 0:PD],
                        pt.rearrange("p (h j) -> p h j", j=PD),
                        bv_sb,
                    )
                for fc in (0, QC):  # q chunk 0, k chunk 0
                    for t5 in range(NT512):
                        emit_qk_tile(fc, t5)

            # remaining q/k work, interleaved into the attention loops
            qk_fill = qk_mm_gen([1, 2, 3])

            fill_state = {"mms": 0, "pause": False}

            def pe_filler():
                """One full-array PE op to keep the HAM clock gate warm.

                The attention matmuls only use half the array (K=64 scores,
                M=65 att@v); measured HW re-throttles the PE to 1.2 GHz
                without a trickle of 128x128 matmuls.  Uses real q/k
                projection work while any is left, then junk matmuls into
                a scratch psum tile.  After each completed projection tile
                (8 matmuls) one call is skipped so the DVE bias-drain can
                free the single-buffered psum slot without stalling the PE.
                """
                if fill_state["pause"]:
                    fill_state["pause"] = False
                    return
                if next(qk_fill, None) is not None:
                    fill_state["mms"] += 1
                    if fill_state["mms"] % 8 == 0:
                        fill_state["pause"] = True
                    return
                if True:
                    pt = qkvps.tile([128, 512], f32, tag="qkv")
                    nc.tensor.matmul(
                        pt[:, 0:256],
                        lhsT=wt_sb[:, 0, 0:128],
                        rhs=xt_sb[:, 0, 0:256],
                        start=True,
                        stop=True,
                    )

            # Deferred epilogues: each n-half's flush/transpose/normalize
            # (and per-head output DMA) is queued and consumed one step per
            # subsequent inner-loop iteration, so the PE never idles at a
            # half boundary waiting on the DVE flush.
            epilogue = deque()

            def epi_step():
                if epilogue:
                    epilogue.popleft()()

            # ---- stage 2: per-head attention ----
            pend = None  # (et, m, o_t, h) -> att@v emitted one iter later
            for h in range(HEADS_PER_CORE):
                hc = h // 2
                hp = (h % 2) * PD  # partition offset of this head's features
                o_sb = osb.tile([65, N], bf16, tag="osb")
                out_t = outp.tile([128, MCH, PD], f32, tag="out")
                for nh in range(2):
                    o_t = ops.tile([65, 1024], f32, tag="O")
                    for m in range(MCH):
                        st = ps.tile([128, 1024], f32, tag="ps")
                        for j in range(2):
                            nsl = nh * 1024 + j * 512
                            nc.tensor.matmul(
                                st[:, j * 512 : (j + 1) * 512],
                                lhsT=kt_sb[hp : hp + PD, hc, m * 128 : (m + 1) * 128],
                                rhs=qt_sb[hp : hp + PD, hc, nsl : nsl + 512],
                                start=True,
                                stop=True,
                            )
                        et = epool.tile([128, 1024], bf16, tag="E")
                        nc.scalar.activation(
                            out=et, in_=st, func=mybir.ActivationFunctionType.Exp
                        )
                        if pend is not None:
                            p_et, p_m, p_ot, p_h = pend
                            for j in range(2):
                                nc.tensor.matmul(
                                    p_ot[:, j * 512 : (j + 1) * 512],
                                    lhsT=vaug_sb[:, p_m, p_h, :],
                                    rhs=p_et[:, j * 512 : (j + 1) * 512],
                                    start=(p_m == 0),
                                    stop=(p_m == MCH - 1),
                                )
                        pend = (et, m, o_t, h)
                        epi_step()
                        pe_filler()
                        if h == 0:
                            pe_filler()

                    def make_epilogue(o_t=o_t, o_sb=o_sb, out_t=out_t, nh=nh, h=h):
                        def flush():
                            nc.vector.tensor_copy(
                                o_sb[:, nh * 1024 : (nh + 1) * 1024], o_t
                            )

                        steps = [flush]
                        for nb in range(nh * 8, nh * 8 + 8):

                            def tnorm(nb=nb):
                                tp = tpp.tile([128, 65], bf16, tag="tp")
                                nc.tensor.transpose(
                                    tp, o_sb[:, nb * 128 : (nb + 1) * 128], ident
                                )
                                rc = small.tile([128, 1], f32, tag="rc")
                                nc.vector.reciprocal(rc, tp[:, PD : PD + 1])
                                nc.vector.tensor_scalar_mul(
                                    out_t[:, nb, :], tp[:, 0:PD], rc
                                )

                            steps.append(tnorm)
                        if nh == 1:

                            def dma_out():
                                nc.sync.dma_start(
                                    out=out_h[h].rearrange("(nb p) j -> p nb j", p=128),
                                    in_=out_t,
                                )

                            steps.append(dma_out)
                        return steps

                    # pend (att@v m=15) is emitted at the start of the next
                    # half; the epilogue steps follow it via the deque.
                    epilogue.extend(make_epilogue())

            # drain: last att@v, then remaining epilogue steps with dummy
            # full-array matmuls between them to keep the PE queue fed
            p_et, p_m, p_ot, p_h = pend
            for j in range(2):
                nc.tensor.matmul(
                    p_ot[:, j * 512 : (j + 1) * 512],
                    lhsT=vaug_sb[:, p_m, p_h, :],
                    rhs=p_et[:, j * 512 : (j + 1) * 512],
                    start=(p_m == 0),
                    stop=(p_m == MCH - 1),
                )
            while epilogue:
                epi_step()
                pe_filler()
    nc.finalize()
    return nc


def _prep_core_inputs(x, Wqkv, bqkv, core):
    b, g = core // 2, core % 2
    xt = np.ascontiguousarray(x[b].T).astype(BF16)  # (D, N)
    wq = Wqkv[g * SHARD_F : (g + 1) * SHARD_F]
    wk = Wqkv[D + g * SHARD_F : D + (g + 1) * SHARD_F]
    wv = Wqkv[2 * D + g * SHARD_F : 2 * D + (g + 1) * SHARD_F]
    wt = np.ascontiguousarray(np.concatenate([wq, wk, wv], axis=0).T).astype(BF16)
    bq = bqkv[g * SHARD_F : (g + 1) * SHARD_F]
    bk = bqkv[D + g * SHARD_F : D + (g + 1) * SHARD_F]
    bv = bqkv[2 * D + g * SHARD_F : 2 * D + (g + 1) * SHARD_F]
    bias_qk = np.concatenate(
        [bq.reshape(4, 128).T, bk.reshape(4, 128).T], axis=1
    ).astype(np.float32)  # (128, 8)
    bias_v = np.broadcast_to(
        bv.reshape(HEADS_PER_CORE, PD), (128, HEADS_PER_CORE, PD)
    ).astype(np.float32)
    return {
        "xt": xt,
        "wt": wt,
        "bias_qk": np.ascontiguousarray(bias_qk),
        "bias_v": np.ascontiguousarray(bias_v),
    }


def kernel(x, Wqkv, bqkv):
    x = np.asarray(x, dtype=np.float32)
    Wqkv = np.asarray(Wqkv, dtype=np.float32)
    bqkv = np.asarray(bqkv, dtype=np.float32)

    if "nc" not in _CACHE:
        _CACHE["nc"] = _build_nc()
    nc = _CACHE["nc"]

    in_maps = [_prep_core_inputs(x, Wqkv, bqkv, c) for c in range(8)]
    res = run_bass_kernel_spmd(nc, in_maps, core_ids=list(range(8)))
    _CACHE["last_result"] = res

    full = np.empty((B, H, N * PD), dtype=np.float32)
    for c in range(8):
        b, g = c // 2, c % 2
        full[b, g * HEADS_PER_CORE : (g + 1) * HEADS_PER_CORE] = res.results[c][
            "out"
        ].reshape(HEADS_PER_CORE, N * PD)
    return full.reshape(B, N, D)
